# revision 17
# baseline (speedup 1.0000x reference)
"""Trainium2 Bass kernel for nn_DecodingLoss_BCEBased (segment_reduce).

Strategy v5 (4 batch-groups x 2 check-halves over 8 NeuronCores):
  - Each core covers 256 batch rows and half the checks (5120 incl. pad).
  - Token table in SBUF holds RAW fp8(e4m3) llrs, one 256B row per token
    (256 batches). The table build is a pure DMA (~5MB) with no compute,
    so gathers start ~25us into the kernel; tanh(0.5*x) is applied by the
    Scalar engine on the *gathered* tiles, overlapping the gather DMAs.
  - dma_gather descriptor generation runs on one Q7 core-pair selected by
    queue_num; gathers rotate across queues 0-3 (num_swdge_queues=4) so
    four core-pairs generate concurrently. fp8 rows halve the DMA-engine
    packet bytes vs bf16 (the 16 SDMA engines are the pacer).
  - The fp8 transpose works on 16-bit granules: partition p of a gathered
    tile receives bytes (2p, 2p+1) of each row = batches 2p and 2p+1,
    interleaved along the free dim (f = 2*i + k, k = batch parity). All
    tree ops are elementwise so the interleave only affects the sgn table
    layout (prepared on host) and the final Ln/accum (two stride-2 slices).
  - Gather idx order is slot-major per chunk so the product-of-8 tree is
    three unit-stride contiguous bf16 multiplies.
  - BCEWithLogits identity: softplus(z) - z*y with z = -2*arctanh(p)
    equals log2 - log(1 - s*p), s = 2y-1. Per check: tree product, * sgn
    (bf16), clamp <= 1-2^-8, ACT Ln(1-x) per batch-parity with accum_out
    doing the sum-over-checks reduction.
  - Observables (8 x 200, padded to 256 with a saturating token fp8=240,
    tanh -> 1.0) run the same path on every core; half-0 cores get
    sgn_obs=0 so they contribute nothing (identical work on all cores).
  - Each core returns S[p, k] = sum ln(1-s*p) for batch bg*256 + 2p + k;
    host: loss = 0.5*(M+K)*log2 - 0.5*mean_b(S_b).
"""
import numpy as np
import ml_dtypes
import concourse.bass as bass
import concourse.tile as tile
from concourse import bacc, mybir
from concourse.bass_utils import run_bass_kernel_spmd

F32 = mybir.dt.float32
BF16 = mybir.dt.bfloat16
F8 = mybir.dt.float8e4
I16 = mybir.dt.int16
AF = mybir.ActivationFunctionType
ALU = mybir.AluOpType
BF = ml_dtypes.bfloat16
F8NP = ml_dtypes.float8_e4m3

P = 128            # SBUF partitions
N_CORES = 8
B, N, M, K = 1024, 20000, 10000, 8
CHK_W, OBS_W = 8, 200

NBG = 4            # batch groups
NK = 2             # batch parity (fp8 byte-pair interleave)
BW = NK * P        # batch rows per core = 256
CHK_CHUNK = 512
N_CHK_HALF = 5120  # padded checks per core (10 chunks)
N_CHUNKS = N_CHK_HALF // CHK_CHUNK
OBS_PW = 256       # obs support padded to pow2

N_STRIPE = (N + P - 1) // P          # 157 data stripes
ONES_ID = N_STRIPE * P               # token in the saturating stripe
N_TOK_PAD = N_STRIPE * P             # 20096 dram rows
TOK_ELEMS = (N_STRIPE + 1) * BW      # fp8 elems per partition in the table

GIDX = CHK_CHUNK * CHK_W             # 4096 idx per chunk gather
N_OBS_IDX = K * OBS_PW               # 2048
ELEM_U16 = BW // 2                   # gather elem_size in int16 units (128)

KMAX = 1.0 - 2.0 ** -8
SATURATE = 240.0                     # fp8 e4m3 max finite; tanh(120) == 1

_NC_CACHE = {}
_TRACE = False  # test.py flips this to get neuron-profile exec_time_ns


def _build_kernel():
    nc = bacc.Bacc("TRN2", target_bir_lowering=False, debug=False,
                   num_devices=N_CORES, num_swdge_queues=4)

    llrsF8 = nc.dram_tensor("llrsF8", [N_TOK_PAD, BW], F8,
                            kind="ExternalInput").ap()
    sgn = nc.dram_tensor("sgn", [P, NK * N_CHK_HALF], BF16,
                         kind="ExternalInput").ap()
    sgn_obs = nc.dram_tensor("sgn_obs", [P, NK * K], BF16,
                             kind="ExternalInput").ap()
    chk_idx = nc.dram_tensor(
        "chk_idx", [P, N_CHK_HALF * CHK_W // 16], I16, kind="ExternalInput").ap()
    obs_idx = nc.dram_tensor(
        "obs_idx", [P, N_OBS_IDX // 16], I16, kind="ExternalInput").ap()
    out = nc.dram_tensor("out", [P, NK], F32, kind="ExternalOutput").ap()

    with tile.TileContext(nc) as tc:
        with (
            tc.tile_pool(name="tok", bufs=1) as tok_pool,
            tc.tile_pool(name="idx", bufs=1) as idx_pool,
            tc.tile_pool(name="g", bufs=5) as g_pool,
            tc.tile_pool(name="th", bufs=2) as th_pool,
            tc.tile_pool(name="gob", bufs=1) as gob_pool,
            tc.tile_pool(name="tree", bufs=1) as tree_pool,
            tc.tile_pool(name="spc", bufs=2) as spc_pool,
            tc.tile_pool(name="acc", bufs=1) as acc_pool,
        ):
            chk_idx_t = idx_pool.tile([P, N_CHK_HALF * CHK_W // 16], I16,
                                      tag="ichk")
            nc.sync.dma_start(chk_idx_t[:], chk_idx)
            obs_idx_t = idx_pool.tile([P, N_OBS_IDX // 16], I16, tag="iobs")
            nc.sync.dma_start(obs_idx_t[:], obs_idx)
            sgo = idx_pool.tile([P, NK * K], BF16, tag="sgo")
            nc.sync.dma_start(sgo[:], sgn_obs)
            sg = idx_pool.tile([P, NK * N_CHK_HALF], BF16, tag="sg")
            nc.sync.dma_start(sg[:], sgn)

            acc = acc_pool.tile([P, NK * (N_CHUNKS + 1)], F32, tag="acc")
            kmax = acc_pool.tile([P, NK * CHK_CHUNK], BF16, tag="kmax")
            nc.vector.memset(kmax[:], KMAX)

            # raw-llr fp8 token table: pure DMA, no compute
            tokT = tok_pool.tile([P, TOK_ELEMS], F8)
            for r in range(0, N_STRIPE, 40):
                ns = min(40, N_STRIPE - r)
                src = llrsF8[bass.ds(r * P, ns * P), :].rearrange(
                    "(rr p) b -> p rr b", p=P)
                dst = tokT[:, bass.ds(r * BW, ns * BW)].rearrange(
                    "p (rr b) -> p rr b", b=BW)
                nc.sync.dma_start(dst, src)
            # saturating stripe for obs padding: tanh(240*0.5) == 1.0
            nc.vector.memset(tokT[:, bass.ds(N_STRIPE * BW, BW)], SATURATE)

            def gather(dst_tile, idxs_ap, n_idx, q):
                # out view [p, k, i]: shape-only (the XBAR write pattern is
                # fixed; base addr + num_idxs + elem_size are what matter).
                nc.gpsimd.dma_gather(
                    out_ap=dst_tile[:].rearrange("p (k i) -> p k i", k=NK),
                    in_ap=tokT[:],
                    idxs_ap=idxs_ap,
                    num_idxs=n_idx,
                    num_idxs_reg=n_idx,
                    elem_size=BW,            # 256 fp8 = 256B per idx
                    transpose=True,
                    single_packet=False,
                    sbuf_tokens_per_rank=P,
                    sbuf_free_dim_per_rank=BW,
                    sbuf_free_dim_pad_per_rank=0,
                    sbuf_byte_offset=0,
                    queue_num=q,
                )

            # observables gather first (queue 3); its compute runs at the end
            gob = gob_pool.tile([P, NK * N_OBS_IDX], F8, tag="gob")
            gather(gob, obs_idx_t[:], N_OBS_IDX, 3)

            # check chunks: gather fp8 -> tanh -> contiguous mult tree
            for c in range(N_CHUNKS):
                g = g_pool.tile([P, NK * GIDX], F8, tag="g", name=f"g{c}")
                gather(g, chk_idx_t[:, bass.ds(c * GIDX // 16, GIDX // 16)],
                       GIDX, c % 4)
                th = th_pool.tile([P, NK * GIDX], BF16, tag="th")
                nc.scalar.activation(th[:], g[:], AF.Tanh, scale=0.5)
                p1 = tree_pool.tile([P, NK * GIDX // 2], BF16, tag="p1")
                nc.vector.tensor_tensor(p1[:], th[:, : NK * GIDX // 2],
                                        th[:, NK * GIDX // 2:], ALU.mult)
                p2 = tree_pool.tile([P, NK * GIDX // 4], BF16, tag="p2")
                nc.vector.tensor_tensor(p2[:], p1[:, : NK * GIDX // 4],
                                        p1[:, NK * GIDX // 4:], ALU.mult)
                p3 = tree_pool.tile([P, NK * CHK_CHUNK], BF16, tag="p3")
                nc.vector.tensor_tensor(p3[:], p2[:, : NK * CHK_CHUNK],
                                        p2[:, NK * CHK_CHUNK:], ALU.mult)
                sp = tree_pool.tile([P, NK * CHK_CHUNK], BF16, tag="sp")
                nc.vector.tensor_tensor(
                    sp[:], p3[:],
                    sg[:, bass.ds(c * NK * CHK_CHUNK, NK * CHK_CHUNK)],
                    ALU.mult)
                spc = spc_pool.tile([P, NK * CHK_CHUNK], BF16, tag="spc")
                nc.vector.tensor_tensor(spc[:], sp[:], kmax[:], ALU.min)
                lnd = tree_pool.tile([P, NK * CHK_CHUNK], BF16, tag="lnd")
                spc3 = spc[:].rearrange("p (i k) -> p k i", k=NK)
                lnd3 = lnd[:].rearrange("p (i k) -> p k i", k=NK)
                for k in range(NK):
                    nc.scalar.activation(
                        lnd3[:, k, :], spc3[:, k, :],
                        AF.Ln, bias=1.0, scale=-1.0,
                        accum_out=acc[:, bass.ds(c * NK + k, 1)])

            # observables compute: tanh -> tree over 256 slots -> sgn -> Ln
            tho = th_pool.tile([P, NK * N_OBS_IDX], BF16, tag="tho")
            nc.scalar.activation(tho[:], gob[:], AF.Tanh, scale=0.5)
            cur = tho[:]
            w = NK * N_OBS_IDX
            lvl = 0
            while w > 2 * NK * K:
                nxt = tree_pool.tile([P, w // 2], BF16, tag=f"ob{lvl}")
                nc.vector.tensor_tensor(nxt[:], cur[:, : w // 2],
                                        cur[:, w // 2:], ALU.mult)
                cur = nxt[:]
                w //= 2
                lvl += 1
            pob = tree_pool.tile([P, NK * K], BF16, tag="pob")
            nc.vector.tensor_tensor(pob[:], cur[:, : NK * K],
                                    cur[:, NK * K:], ALU.mult)
            nc.vector.tensor_tensor(pob[:], pob[:], sgo[:], ALU.mult)
            nc.vector.tensor_tensor(pob[:], pob[:], kmax[:, : NK * K], ALU.min)
            lno = tree_pool.tile([P, NK * K], BF16, tag="lno")
            pob3 = pob[:].rearrange("p (i k) -> p k i", k=NK)
            lno3 = lno[:].rearrange("p (i k) -> p k i", k=NK)
            for k in range(NK):
                nc.scalar.activation(
                    lno3[:, k, :], pob3[:, k, :],
                    AF.Ln, bias=1.0, scale=-1.0,
                    accum_out=acc[:, bass.ds(NK * N_CHUNKS + k, 1)])

            s_t = acc_pool.tile([P, NK], F32, tag="st")
            accv = acc[:].rearrange("p (c k) -> p k c", k=NK)
            nc.vector.tensor_reduce(s_t[:], accv, mybir.AxisListType.X,
                                    ALU.add)
            nc.sync.dma_start(out, s_t[:])

    nc.compile()
    return nc


def _get_nc():
    if "nc" not in _NC_CACHE:
        _NC_CACHE["nc"] = _build_kernel()
    return _NC_CACHE["nc"]


def _wrap_idx(flat):
    # dma_gather index layout: unwrapped[s*16+p] = tile[p, s], replicated
    # across the eight 16-partition groups
    n = flat.shape[0]
    w = flat.reshape(n // 16, 16).T.astype(np.int16)
    return np.tile(w, (8, 1))


def kernel(llrs, syndromes, observables, chk_cols, obs_cols):
    llrs = np.asarray(llrs, dtype=np.float32)
    syndromes = np.asarray(syndromes, dtype=np.float32)
    observables = np.asarray(observables, dtype=np.float32)
    chk_cols = np.asarray(chk_cols)
    obs_cols = np.asarray(obs_cols)

    nc = _get_nc()

    # token-major llrs, fp8: [N_TOK_PAD, B]
    llrsT = np.zeros((N_TOK_PAD, B), F8NP)
    llrsT[:N] = llrs.T

    # sgn, padded to N_CHK_HALF per half, interleaved f = i*2 + k for
    # batch b' = 2p + k
    sgn_full = np.zeros((B, 2 * N_CHK_HALF), BF)
    sgn_full[:, :M] = (2.0 * syndromes - 1.0)
    sgn_obs_full = (2.0 * observables - 1.0).astype(BF)

    # check idx, slot-major per 512-chunk: idx[c*4096 + s*512 + i]
    chk_pad = np.zeros((2 * N_CHK_HALF, CHK_W), np.int64)
    chk_pad[:M] = chk_cols

    def chk_idx_half(h):
        cc = chk_pad[h * N_CHK_HALF:(h + 1) * N_CHK_HALF]
        cc = cc.reshape(N_CHUNKS, CHK_CHUNK, CHK_W).transpose(0, 2, 1)
        return _wrap_idx(cc.reshape(-1))

    chk_idx_w = [chk_idx_half(0), chk_idx_half(1)]

    # obs idx, slot-major: idx[s*8 + k], slots >= 200 -> saturating token
    op = np.full((K, OBS_PW), ONES_ID, np.int64)
    op[:, :OBS_W] = obs_cols
    obs_idx_w = _wrap_idx(op.T.reshape(-1))

    in_maps = []
    for core in range(N_CORES):
        bg, half = core // 2, core % 2
        bsl = slice(bg * BW, (bg + 1) * BW)
        # sgn slice -> [p, c, i, k] -> [128, NK*N_CHK_HALF]
        v = sgn_full[bsl, half * N_CHK_HALF:(half + 1) * N_CHK_HALF]
        v = v.reshape(P, NK, N_CHUNKS, CHK_CHUNK).transpose(0, 2, 3, 1)
        so = sgn_obs_full[bsl].reshape(P, NK, K).transpose(0, 2, 1)
        if half == 0:
            so = np.zeros_like(so)
        in_maps.append({
            "llrsF8": np.ascontiguousarray(llrsT[:, bsl]),
            "sgn": np.ascontiguousarray(v.reshape(P, NK * N_CHK_HALF)),
            "sgn_obs": np.ascontiguousarray(so.reshape(P, NK * K)),
            "chk_idx": chk_idx_w[half],
            "obs_idx": obs_idx_w,
        })

    res = run_bass_kernel_spmd(nc, in_maps, core_ids=list(range(N_CORES)),
                               trace=_TRACE)
    _NC_CACHE["exec_time_ns"] = res.exec_time_ns
    # S[bg*256 + 2p + k] = sum over both halves
    S = np.zeros((NBG, P, NK), np.float64)
    for core in range(N_CORES):
        bg = core // 2
        S[bg] += res.results[core]["out"].astype(np.float64)  # [p, k]
    S = S.reshape(B)
    loss_b = 0.5 * (M + K) * np.log(2.0) - 0.5 * S
    return np.float32(loss_b.mean())
